# revision 9
# baseline (speedup 1.0000x reference)
"""Trainium2 Bass kernel for nn_GATLinNet (3-layer GAT + Linear residual net).

Self-contained: takes FULL inputs, shards nodes across 8 NeuronCores
(dst-sharded graph parallelism), runs one SPMD NEFF, returns FULL output.

Design:
  - Node tables (xl features + attention-src terms, bf16, head-minor layout)
    built per-layer on each core for its own node shard, then AllGathered.
  - Edge aggregation per core over its incoming edges, dst-sorted, in
    128-edge blocks: per-edge source rows fetched with dma_gather (int16
    indices -> node table split in two row-halves), per-block one-hot P
    matrices (is_equal vs iota) turn segment-sum into TensorE matmuls
    accumulated in PSUM per 128-dst-node tile.
  - Per-edge softmax weights w = exp(leaky_relu(as[src] + ad[dst])) built
    from gathered as (bf16 hi/lo pair) and ad via P^T matmul against the
    SBUF-resident per-tile ad vector. Denominators ride along the scatter
    matmul as a per-head ones column; normalization at tile close.
"""

import math

import numpy as np
import ml_dtypes

import concourse.bacc as bacc
import concourse.bass as bass
import concourse.mybir as mybir
import concourse.tile as tile
from concourse.bass_utils import run_bass_kernel_spmd
from concourse.library_config import mlp as mlp_lib

F32 = mybir.dt.float32
BF16 = mybir.dt.bfloat16
I16 = mybir.dt.int16

H = 4
HID = 64
IN = 128
D1 = H * HID  # 256
NEG = 0.2
TILE = 128


class Cfg:
    def __init__(self, n_real=50000, ncores=8, nper=6272):
        self.n_real = n_real
        self.ncores = ncores
        self.nper = nper                      # nodes per core, multiple of 128
        assert nper % TILE == 0
        self.npad = ncores * nper
        assert self.npad % (2 * TILE) == 0
        self.half = self.npad // 2            # table row split for int16 idx
        assert self.half % TILE == 0 and self.half - 1 <= 32767
        self.ntiles = nper // TILE


def _bf(x):
    return np.ascontiguousarray(np.asarray(x)).astype(ml_dtypes.bfloat16)


def _f32(x):
    return np.ascontiguousarray(np.asarray(x, dtype=np.float32))


def prep_graph(edge_index, cfg):
    """Host-side index-only preprocessing. Returns per-core arrays + static
    structure (shared across cores, so the SPMD program is uniform)."""
    n, nc_, nt = cfg.n_real, cfg.ncores, cfg.ntiles
    src = np.concatenate([edge_index[0].astype(np.int64),
                          np.arange(n, dtype=np.int64)])
    dst = np.concatenate([edge_index[1].astype(np.int64),
                          np.arange(n, dtype=np.int64)])

    groups = []   # per core: dict (t, h) -> (src_ids, dst_loc)
    cnt = np.zeros((nc_, nt, 2), np.int64)
    for c in range(nc_):
        lo = c * cfg.nper
        m = (dst >= lo) & (dst < lo + cfg.nper)
        s, d = src[m], dst[m] - lo
        t_id = d // TILE
        half = (s >= cfg.half).astype(np.int64)
        key = t_id * 2 + half
        order = np.argsort(key, kind="stable")
        s, d, t_id, half, key = s[order], d[order], t_id[order], half[order], key[order]
        # group boundaries
        gmap = {}
        uniq, starts = np.unique(key, return_index=True)
        starts = list(starts) + [len(key)]
        for i, k in enumerate(uniq):
            sl = slice(starts[i], starts[i + 1])
            gmap[(int(k) // 2, int(k) % 2)] = (s[sl], d[sl])
            cnt[c, int(k) // 2, int(k) % 2] = starts[i + 1] - starts[i]
        groups.append(gmap)

    cmax = cnt.max(axis=0)                               # [nt, 2]
    nv = np.ceil(cmax / TILE).astype(np.int64)           # blocks per (t, half)
    ba, bb = int(nv[:, 0].max()), int(nv[:, 1].max())    # capacities
    nblk = nt * (ba + bb)

    idx_arrs = []
    dstr_arrs = []
    for c in range(nc_):
        gmap = groups[c]
        idxA = np.zeros((nt, ba * TILE), np.int64)
        idxB = np.zeros((nt, bb * TILE), np.int64)
        dloc = np.full((nt, ba + bb, TILE), 200.0, np.float32)
        for t in range(nt):
            for h, (idx, cap) in enumerate(((idxA, ba), (idxB, bb))):
                s_d = gmap.get((t, h))
                if s_d is None:
                    continue
                s, d = s_d
                k = len(s)
                loc = s - (cfg.half if h else 0)
                idx[t, :k] = loc
                boff = 0 if h == 0 else ba
                for b in range(int(nv[t, h])):
                    sl = slice(b * TILE, min((b + 1) * TILE, k))
                    nvalid = sl.stop - sl.start
                    if nvalid > 0:
                        dloc[t, boff + b, :nvalid] = (d[sl] - t * TILE)
                    if nvalid < TILE:
                        dloc[t, boff + b, nvalid:] = 200.0
        # wrap indices: slot i -> [i % 16, i // 16], replicate x8 partitions
        def wrap(arr, cap):
            w = arr.reshape(nt, cap * 8, 16).transpose(0, 2, 1)  # [nt,16,cap*8]
            w = np.concatenate([w[t] for t in range(nt)], axis=1)  # [16, nt*cap*8]
            return np.tile(w, (8, 1)).astype(np.int16)
        idx_arrs.append((wrap(idxA, ba), wrap(idxB, bb)))
        # dstr: [128, nt*(ba+bb)]  value at [p, t*(ba+bb)+b] = dloc[t, b, p]
        dstr = dloc.reshape(nt * (ba + bb), TILE).T
        dstr_arrs.append(_f32(dstr))

    nv = nv.astype(int)
    return dict(nv=nv, ba=ba, bb=bb, nblk=nblk,
                idx=idx_arrs, dstr=dstr_arrs)


# ---------------------------------------------------------------------------
# program builder
# ---------------------------------------------------------------------------

def build_program(cfg, nv, ba, bb, debug=False):
    nt = cfg.ntiles
    nc = bacc.Bacc("TRN2")

    # ---- external inputs --------------------------------------------------
    ein = {}

    def inp(name, shape, dt):
        ein[name] = nc.dram_tensor(name, list(shape), dt, kind="ExternalInput")
        return ein[name]

    x_own = inp("x_own", (cfg.nper, IN), F32)
    idxA = inp("idxA", (128, nt * ba * 8), I16)
    idxB = inp("idxB", (128, nt * bb * 8), I16)
    dstr = inp("dstr", (128, nt * (ba + bb)), F32)
    w_in, lw_in, atts_in, attd_in, cb_in, lb_in = [], [], [], [], [], []
    for L, (fin, fout) in enumerate(((IN, D1), (D1, D1), (D1, H))):
        kk = fin // 128
        w_in.append(inp(f"W{L}", (128, kk, fout), BF16))
        lout = fout if L < 2 else 1
        lw_in.append(inp(f"lW{L}", (128, kk, lout), BF16))
        aw = fout
        atts_in.append(inp(f"attS{L}", (128, aw), F32))
        attd_in.append(inp(f"attD{L}", (128, aw), F32))
        cw = D1 if L < 2 else 1
        cb_in.append(inp(f"cb{L}", (128, cw), F32))
        lb_in.append(inp(f"lb{L}", (128, cw), F32))
    iota_in = inp("iota", (128, 128), BF16)
    identb_in = inp("identb", (128, 128), BF16)
    identf_in = inp("identf", (128, 128), F32)

    out_ext = nc.dram_tensor("out", [cfg.nper, 1], F32, kind="ExternalOutput")

    # table widths per layer
    TW = [384, 384, 128]     # row stride (bf16 elems)
    RW = [260, 260, 8]       # scatter matmul width (msg + ones)
    XW = [256, 256, 4]       # xl width
    HI0 = [260, 260, 8]      # as_hi col
    LO0 = [264, 264, 12]
    USED = [268, 268, 16]

    cc_groups = [list(range(cfg.ncores))]

    with tile.TileContext(nc) as tc:
        import contextlib
        stack = contextlib.ExitStack()
        sb = stack.enter_context(tc.tile_pool(name="sb", bufs=3))
        sbc = stack.enter_context(tc.tile_pool(name="sbc", bufs=1))
        ps_big = stack.enter_context(tc.tile_pool(name="psbig", bufs=3, space="PSUM"))
        ps_sm = stack.enter_context(tc.tile_pool(name="pssm", bufs=2, space="PSUM"))
        ps_agg = stack.enter_context(tc.tile_pool(name="psagg", bufs=2, space="PSUM"))
        dram = stack.enter_context(tc.tile_pool(name="dram", bufs=1, space="DRAM"))

        nc.gpsimd.load_library(mlp_lib)

        # ---- persistent SBUF constants -----------------------------------
        def load_const(name, ap):
            t = sbc.tile(list(ap.shape), ap.dtype, name=name)
            nc.sync.dma_start(t[:], ap[:])
            return t

        W_sb = [load_const(f"W{L}sb", w_in[L]) for L in range(3)]
        lW_sb = [load_const(f"lW{L}sb", lw_in[L]) for L in range(3)]
        attS_sb = [load_const(f"attS{L}sb", atts_in[L]) for L in range(3)]
        attD_sb = [load_const(f"attD{L}sb", attd_in[L]) for L in range(3)]
        iota_sb = load_const("iotasb", iota_in)
        identb_sb = load_const("identbsb", identb_in)
        identf_sb = load_const("identfsb", identf_in)
        idxA_sb = load_const("idxAsb", idxA)
        idxB_sb = load_const("idxBsb", idxB)
        dstr_sb = load_const("dstrsb", dstr)
        cbl_sb = []
        for L in range(3):
            cw = D1 if L < 2 else 1
            cbt = sbc.tile([128, cw], F32, name=f"cbl{L}")
            c0 = load_const(f"cb{L}t", cb_in[L])
            l0 = load_const(f"lb{L}t", lb_in[L])
            nc.vector.tensor_add(cbt[:], c0[:], l0[:])
            cbl_sb.append(cbt)
        # per-tile ad storage (bf16) for all layers' current use
        ad_all = sbc.tile([128, nt * H], BF16, name="adall")

        # ---- DRAM scratch -------------------------------------------------
        t_own = [dram.tile([cfg.nper, TW[L]], BF16, name=f"town{L}") for L in range(3)]
        t_full = [dram.tile([cfg.npad, TW[L]], BF16, name=f"tfull{L}") for L in range(3)]
        if debug:
            h_own = [nc.dram_tensor(f"hdbg{L}", [cfg.nper, D1], BF16,
                                    kind="ExternalOutput") for L in range(2)]
        else:
            h_own = [dram.tile([cfg.nper, D1], BF16, name=f"hown{L}") for L in range(2)]
        lin_dram = dram.tile([cfg.nper, D1], F32, name="lindram")
        lin2_dram = dram.tile([cfg.nper, 1], F32, name="lin2dram")

        # ------------------------------------------------------------------
        def phase_a(L):
            fin = IN if L == 0 else D1
            kk = fin // 128
            fout = XW[L]
            lout = D1 if L < 2 else 1
            for t in range(nt):
                rows = slice(t * TILE, (t + 1) * TILE)
                if L == 0:
                    xin = sb.tile([128, IN], F32, tag="axin")
                    nc.sync.dma_start(xin[:], x_own[rows, :])
                    xt_ps = ps_big.tile([128, 128], F32, tag="bigps")
                    nc.tensor.transpose(xt_ps[:], xin[:], identf_sb[:])
                    actT = sb.tile([128, 1, 128], BF16, tag="actT")
                    nc.vector.tensor_copy(actT[:, 0, :], xt_ps[:])
                else:
                    hsrc = h_own[L - 1]
                    actT = sb.tile([128, 2, 128], BF16, tag="actT")
                    for k in range(2):
                        nc.sync.dma_start(
                            actT[:, k, :],
                            hsrc[rows, k * 128:(k + 1) * 128],
                            transpose=True,
                        )
                xl_ps = ps_big.tile([128, fout], F32, tag="bigps")
                lin_ps = ps_sm.tile([128, lout], F32, tag="smps")
                for k in range(kk):
                    nc.tensor.matmul(xl_ps[:], lhsT=actT[:, k, :], rhs=W_sb[L][:, k, :],
                                     start=(k == 0), stop=(k == kk - 1))
                for k in range(kk):
                    nc.tensor.matmul(lin_ps[:], lhsT=actT[:, k, :], rhs=lW_sb[L][:, k, :],
                                     start=(k == 0), stop=(k == kk - 1))
                # attention terms
                aa = sb.tile([128, 2 * H], F32, tag="aa")
                if L < 2:
                    for j, att in enumerate((attS_sb[L], attD_sb[L])):
                        prod = sb.tile([128, fout], F32, tag="prod")
                        nc.vector.tensor_mul(prod[:], xl_ps[:], att[:])
                        nc.vector.tensor_reduce(
                            aa[:, j * H:(j + 1) * H],
                            prod[:].rearrange("p (h j) -> p h j", h=H),
                            axis=mybir.AxisListType.X, op=mybir.AluOpType.add)
                else:
                    nc.vector.tensor_mul(aa[:, 0:H], xl_ps[:], attS_sb[L][:])
                    nc.vector.tensor_mul(aa[:, H:2 * H], xl_ps[:], attD_sb[L][:])
                nc.vector.tensor_copy(ad_all[:, t * H:(t + 1) * H], aa[:, H:2 * H])
                # table tile (full row width; pad cols zeroed for the
                # allgather + gather path)
                tbl = sb.tile([128, TW[L]], BF16, tag="tbl")
                nc.vector.memset(tbl[:, USED[L]:TW[L]], 0.0)
                if L < 2:
                    nc.vector.tensor_copy(
                        tbl[:, 0:XW[L]].rearrange("p (j h) -> p j h", h=H),
                        xl_ps[:].rearrange("p (h j) -> p j h", j=64),
                    )
                else:
                    nc.vector.tensor_copy(tbl[:, 0:H], xl_ps[:])
                nc.vector.memset(tbl[:, XW[L]:XW[L] + H], 1.0)
                # as hi/lo
                nc.scalar.activation(tbl[:, HI0[L]:HI0[L] + H], aa[:, 0:H],
                                     mybir.ActivationFunctionType.Copy)
                hi_f = sb.tile([128, H], F32, tag="hif")
                nc.scalar.activation(hi_f[:], tbl[:, HI0[L]:HI0[L] + H],
                                     mybir.ActivationFunctionType.Copy)
                nc.vector.tensor_sub(tbl[:, LO0[L]:LO0[L] + H], aa[:, 0:H], hi_f[:])
                nc.sync.dma_start(t_own[L][rows, :], tbl[:])
                # lin
                lin_sb = sb.tile([128, lout], F32, tag="linsb")
                nc.vector.tensor_copy(lin_sb[:], lin_ps[:])
                tgt = lin_dram if L < 2 else lin2_dram
                nc.sync.dma_start(tgt[rows, :], lin_sb[:])
            # publish table
            nc.gpsimd.collective_compute(
                "AllGather", mybir.AluOpType.bypass, replica_groups=cc_groups,
                ins=[t_own[L][:, :].opt()], outs=[t_full[L][:, :].opt()])

        # ------------------------------------------------------------------
        def edge_phase(L):
            tw, rw, used = TW[L], RW[L], USED[L]
            for t in range(nt):
                agg = ps_agg.tile([128, rw], F32, tag="agg")
                n_mm = int(nv[t, 0]) + int(nv[t, 1])
                mm_i = 0
                for hh, (idx_sb, cap, boff) in enumerate(
                        ((idxA_sb, ba, 0), (idxB_sb, bb, ba))):
                    v = int(nv[t, hh])
                    if v == 0:
                        continue
                    rlo = hh * cfg.half
                    gath = sb.tile([128, cap, tw], BF16, tag=f"gath{L % 2}")
                    nc.gpsimd.dma_gather(
                        gath[:, 0:v, :],
                        t_full[L][rlo:rlo + cfg.half, :],
                        idx_sb[:, t * cap * 8: t * cap * 8 + v * 8],
                        v * TILE, v * TILE, tw, single_packet=False)
                    # one-hot P and its transpose
                    p_all = sb.tile([128, cap, 128], BF16, tag="pall")
                    pt_sb = sb.tile([128, cap, 128], BF16, tag="ptsb")
                    blk0 = t * (ba + bb) + boff
                    for g in range(0, v, 4):
                        nb = min(4, v - g)
                        pt_ps = ps_big.tile([128, 512], BF16, tag="bigps")
                        for b in range(g, g + nb):
                            nc.vector.tensor_scalar(
                                out=p_all[:, b, :], in0=iota_sb[:],
                                scalar1=dstr_sb[:, blk0 + b: blk0 + b + 1],
                                scalar2=None, op0=mybir.AluOpType.is_equal)
                            nc.tensor.transpose(
                                pt_ps[:, (b - g) * 128:(b - g + 1) * 128],
                                p_all[:, b, :], identb_sb[:])
                        nc.scalar.activation(
                            pt_sb[:].rearrange("p c j -> p (c j)")[:, g * 128:(g + nb) * 128],
                            pt_ps[:, 0:nb * 128],
                            mybir.ActivationFunctionType.Copy)
                    # per-edge ad via PT @ ad_tile
                    ad_ps = ps_sm.tile([128, cap * H], F32, tag="smps")
                    for b in range(v):
                        nc.tensor.matmul(
                            ad_ps[:, b * H:(b + 1) * H], lhsT=pt_sb[:, b, :],
                            rhs=ad_all[:, t * H:(t + 1) * H],
                            start=True, stop=True)
                    # logits -> w (bf16)
                    e_sb = sb.tile([128, cap * H], F32, tag="esb")
                    hi_ap = gath[:, 0:v, HI0[L]:HI0[L] + H]
                    lo_ap = gath[:, 0:v, LO0[L]:LO0[L] + H]
                    nc.vector.tensor_add(
                        e_sb[:, 0:v * H].rearrange("p (c h) -> p c h", h=H),
                        hi_ap, lo_ap)
                    nc.vector.scalar_tensor_tensor(
                        out=e_sb[:, 0:v * H], in0=e_sb[:, 0:v * H], scalar=1.0,
                        in1=ad_ps[:, 0:v * H],
                        op0=mybir.AluOpType.mult, op1=mybir.AluOpType.add)
                    lr_sb = sb.tile([128, cap * H], F32, tag="lrsb")
                    nc.vector.tensor_scalar(
                        out=lr_sb[:, 0:v * H], in0=e_sb[:, 0:v * H],
                        scalar1=NEG, scalar2=None, op0=mybir.AluOpType.mult)
                    nc.vector.tensor_max(lr_sb[:, 0:v * H], lr_sb[:, 0:v * H],
                                         e_sb[:, 0:v * H])
                    w_bf = sb.tile([128, cap, H], BF16, tag="wbf")
                    nc.scalar.activation(
                        w_bf[:, 0:v, :].rearrange("p c h -> p (c h)"),
                        lr_sb[:, 0:v * H], mybir.ActivationFunctionType.Exp)
                    # msg + scatter
                    for g in range(0, v, 4):
                        nb = min(4, v - g)
                        msg = sb.tile([128, 4, rw], BF16, tag="msg")
                        w_ap = bass.AP(
                            w_bf.tensor, w_bf[:, g, :].offset,
                            [w_bf[:].ap[0], [H, nb], [0, rw // H], [1, H]],
                        )
                        nc.vector.tensor_tensor(
                            out=msg[:, 0:nb, :], in0=gath[:, g:g + nb, 0:rw],
                            in1=w_ap, op=mybir.AluOpType.mult)
                        for b in range(nb):
                            nc.tensor.matmul(
                                agg[:], lhsT=p_all[:, g + b, :], rhs=msg[:, b, :],
                                start=(mm_i == 0), stop=(mm_i == n_mm - 1),
                                skip_group_check=True)
                            mm_i += 1
                # ---- tile close ----
                rows = slice(t * TILE, (t + 1) * TILE)
                den_b = sb.tile([128, H], F32, tag="denb")
                nc.vector.tensor_scalar_add(den_b[:], agg[:, XW[L]:XW[L] + H], 1e-12)
                recip = sb.tile([128, H], F32, tag="recip")
                nc.vector.reciprocal(recip[:], den_b[:])
                if L < 2:
                    lin_sb = sb.tile([128, D1], F32, tag="linrd")
                    nc.sync.dma_start(lin_sb[:], lin_dram[rows, :])
                    gat = sb.tile([128, D1], F32, tag="gatsb")
                    # out head-major <- agg head-minor, scaled by recip
                    agg_hm = bass.AP(agg.tensor, agg[:].offset,
                                     [agg[:].ap[0], [1, H], [H, 64]])
                    rec_b = bass.AP(recip.tensor, recip[:].offset,
                                    [recip[:].ap[0], [1, H], [0, 64]])
                    nc.vector.tensor_tensor(out=gat[:].rearrange("p (h j) -> p h j", h=H),
                                            in0=agg_hm, in1=rec_b,
                                            op=mybir.AluOpType.mult)
                    gsum = sb.tile([128, D1], F32, tag="gsum")
                    nc.vector.scalar_tensor_tensor(
                        out=gsum[:], in0=gat[:], scalar=1.0, in1=lin_sb[:],
                        op0=mybir.AluOpType.mult, op1=mybir.AluOpType.add)
                    nc.vector.tensor_add(gsum[:], gsum[:], cbl_sb[L][:])
                    # ELU: relu(g) + exp(-relu(-g)) - 1
                    r1 = sb.tile([128, D1], F32, tag="r1")
                    nc.scalar.activation(r1[:], gsum[:],
                                         mybir.ActivationFunctionType.Relu)
                    r2 = sb.tile([128, D1], F32, tag="r2")
                    nc.scalar.activation(r2[:], gsum[:],
                                         mybir.ActivationFunctionType.Relu,
                                         scale=-1.0)
                    r3 = sb.tile([128, D1], F32, tag="r3")
                    nc.scalar.activation(r3[:], r2[:],
                                         mybir.ActivationFunctionType.Exp,
                                         scale=-1.0)
                    h_bf = sb.tile([128, D1], BF16, tag="hbf")
                    nc.vector.scalar_tensor_tensor(
                        out=h_bf[:], in0=r1[:], scalar=-1.0, in1=r3[:],
                        op0=mybir.AluOpType.add, op1=mybir.AluOpType.add)
                    nc.sync.dma_start(h_own[L][rows, :], h_bf[:])
                else:
                    lin_sb = sb.tile([128, 1], F32, tag="linrd2")
                    nc.sync.dma_start(lin_sb[:], lin2_dram[rows, :])
                    q = sb.tile([128, H], F32, tag="qsb")
                    nc.vector.tensor_mul(q[:], agg[:, 0:H], recip[:])
                    msum = sb.tile([128, 1], F32, tag="msum")
                    nc.vector.tensor_reduce(msum[:], q[:],
                                            axis=mybir.AxisListType.X,
                                            op=mybir.AluOpType.add)
                    o1 = sb.tile([128, 1], F32, tag="o1")
                    nc.vector.scalar_tensor_tensor(
                        out=o1[:], in0=msum[:], scalar=1.0 / H, in1=lin_sb[:],
                        op0=mybir.AluOpType.mult, op1=mybir.AluOpType.add)
                    nc.vector.tensor_add(o1[:], o1[:], cbl_sb[L][:])
                    nc.sync.dma_start(out_ext[rows, :], o1[:])

        for L in range(3):
            phase_a(L)
            edge_phase(L)

        stack.close()

    nc.compile()
    return nc, ein


# ---------------------------------------------------------------------------
# host wrapper
# ---------------------------------------------------------------------------

def _make_inputs(cfg, prep, core, x, weights):
    (cW0, cas0, cad0, cb0, lW0, lb0,
     cW1, cas1, cad1, cb1, lW1, lb1,
     cW2, cas2, cad2, cb2, lW2, lb2) = weights
    nper = cfg.nper
    xp = np.zeros((nper, IN), np.float32)
    lo = core * nper
    hi = min((core + 1) * nper, cfg.n_real)
    if hi > lo:
        xp[0:hi - lo] = x[lo:hi]
    idxA, idxB = prep["idx"][core]

    def stackk(w):
        w = _f32(w)
        kk = w.shape[0] // 128
        return _bf(np.stack([w[k * 128:(k + 1) * 128] for k in range(kk)], axis=1))

    rep = lambda v: np.broadcast_to(_f32(v).reshape(1, -1), (128, _f32(v).size)).copy()
    d = {
        "x_own": xp,
        "idxA": idxA, "idxB": idxB, "dstr": prep["dstr"][core],
        "W0": stackk(cW0), "W1": stackk(cW1), "W2": stackk(cW2),
        "lW0": stackk(lW0), "lW1": stackk(lW1), "lW2": stackk(lW2),
        "attS0": rep(cas0.reshape(-1)), "attD0": rep(cad0.reshape(-1)),
        "attS1": rep(cas1.reshape(-1)), "attD1": rep(cad1.reshape(-1)),
        "attS2": rep(cas2.reshape(-1)), "attD2": rep(cad2.reshape(-1)),
        "cb0": rep(cb0), "lb0": rep(lb0),
        "cb1": rep(cb1), "lb1": rep(lb1),
        "cb2": rep(cb2), "lb2": rep(lb2),
        "iota": _bf(np.broadcast_to(np.arange(128, dtype=np.float32), (128, 128))),
        "identb": _bf(np.eye(128, dtype=np.float32)),
        "identf": _f32(np.eye(128, dtype=np.float32)),
    }
    return d


_CACHE = {}
LAST_EXEC_NS = None
LAST_PROFILE = None


def kernel(x, edge_index, batch,
           cW0, cas0, cad0, cb0, lW0, lb0,
           cW1, cas1, cad1, cb1, lW1, lb1,
           cW2, cas2, cad2, cb2, lW2, lb2):
    cfg = Cfg(n_real=x.shape[0], ncores=8,
              nper=math.ceil(x.shape[0] / 8 / TILE) * TILE)
    x = _f32(x)
    edge_index = np.asarray(edge_index)
    prep = prep_graph(edge_index, cfg)
    key = ("prog", cfg.n_real, prep["ba"], prep["bb"], prep["nv"].tobytes())
    if key not in _CACHE:
        _CACHE[key] = build_program(cfg, prep["nv"], prep["ba"], prep["bb"])
    nc, _ = _CACHE[key]
    weights = (cW0, cas0, cad0, cb0, lW0, lb0,
               cW1, cas1, cad1, cb1, lW1, lb1,
               cW2, cas2, cad2, cb2, lW2, lb2)
    in_maps = [_make_inputs(cfg, prep, c, x, weights) for c in range(cfg.ncores)]
    import os
    trace = bool(os.environ.get("KERNEL_PROFILE"))
    res = run_bass_kernel_spmd(nc, in_maps, list(range(cfg.ncores)), trace=trace)
    global LAST_EXEC_NS, LAST_PROFILE
    LAST_EXEC_NS = getattr(res, "exec_time_ns", None)
    LAST_PROFILE = getattr(res, "profile_json", None)
    out = np.concatenate([res.results[c]["out"] for c in range(cfg.ncores)], axis=0)
    return out[:cfg.n_real].astype(np.float32)


# revision 11
# speedup vs baseline: 15.1695x; 15.1695x over previous
"""Trainium2 Bass kernel for nn_GATLinNet (3-layer GAT + Linear residual net).

Self-contained: takes FULL inputs, shards nodes across 8 NeuronCores
(dst-sharded graph parallelism), runs one SPMD NEFF, returns FULL output.

Design:
  - Node tables (xl features + attention-src terms, bf16, head-minor layout)
    built per-layer on each core for its own node shard, then AllGathered.
  - Edge aggregation per core over its incoming edges, dst-sorted, in
    128-edge blocks: per-edge source rows fetched with dma_gather (int16
    indices -> node table split in two row-halves), per-block one-hot P
    matrices (is_equal vs iota) turn segment-sum into TensorE matmuls
    accumulated in PSUM per 128-dst-node tile.
  - Per-edge softmax weights w = exp(leaky_relu(as[src] + ad[dst])) built
    from gathered as (bf16 hi/lo pair) and ad via P^T matmul against the
    SBUF-resident per-tile ad vector. Denominators ride along the scatter
    matmul as a per-head ones column; normalization at tile close.
"""

import math

import numpy as np
import ml_dtypes

import concourse.bacc as bacc
import concourse.bass as bass
import concourse.mybir as mybir
import concourse.tile as tile
from concourse.bass_utils import run_bass_kernel_spmd
from concourse.library_config import mlp as mlp_lib

F32 = mybir.dt.float32
BF16 = mybir.dt.bfloat16
I16 = mybir.dt.int16

H = 4
HID = 64
IN = 128
D1 = H * HID  # 256
NEG = 0.2
TILE = 128


class Cfg:
    def __init__(self, n_real=50000, ncores=8, nper=6272):
        self.n_real = n_real
        self.ncores = ncores
        self.nper = nper                      # nodes per core, multiple of 128
        assert nper % TILE == 0
        self.npad = ncores * nper
        assert self.npad % (2 * TILE) == 0
        self.half = self.npad // 2            # table row split for int16 idx
        assert self.half % TILE == 0 and self.half - 1 <= 32767
        self.ntiles = nper // TILE


def _bf(x):
    return np.ascontiguousarray(np.asarray(x)).astype(ml_dtypes.bfloat16)


def _f32(x):
    return np.ascontiguousarray(np.asarray(x, dtype=np.float32))


def prep_graph(edge_index, cfg):
    """Host-side index-only preprocessing. Returns per-core arrays + static
    structure (shared across cores, so the SPMD program is uniform)."""
    n, nc_, nt = cfg.n_real, cfg.ncores, cfg.ntiles
    src = np.concatenate([edge_index[0].astype(np.int64),
                          np.arange(n, dtype=np.int64)])
    dst = np.concatenate([edge_index[1].astype(np.int64),
                          np.arange(n, dtype=np.int64)])

    groups = []   # per core: dict (t, h) -> (src_ids, dst_loc)
    cnt = np.zeros((nc_, nt, 2), np.int64)
    for c in range(nc_):
        lo = c * cfg.nper
        m = (dst >= lo) & (dst < lo + cfg.nper)
        s, d = src[m], dst[m] - lo
        t_id = d // TILE
        half = (s >= cfg.half).astype(np.int64)
        key = t_id * 2 + half
        order = np.argsort(key, kind="stable")
        s, d, t_id, half, key = s[order], d[order], t_id[order], half[order], key[order]
        # group boundaries
        gmap = {}
        uniq, starts = np.unique(key, return_index=True)
        starts = list(starts) + [len(key)]
        for i, k in enumerate(uniq):
            sl = slice(starts[i], starts[i + 1])
            gmap[(int(k) // 2, int(k) % 2)] = (s[sl], d[sl])
            cnt[c, int(k) // 2, int(k) % 2] = starts[i + 1] - starts[i]
        groups.append(gmap)

    cmax = cnt.max(axis=0)                               # [nt, 2]
    nv = np.ceil(cmax / TILE).astype(np.int64)           # blocks per (t, half)
    ba, bb = int(nv[:, 0].max()), int(nv[:, 1].max())    # capacities
    nblk = nt * (ba + bb)

    idx_arrs = []
    dstr_arrs = []
    for c in range(nc_):
        gmap = groups[c]
        idxA = np.zeros((nt, ba * TILE), np.int64)
        idxB = np.zeros((nt, bb * TILE), np.int64)
        dloc = np.full((nt, ba + bb, TILE), 200.0, np.float32)
        for t in range(nt):
            for h, (idx, cap) in enumerate(((idxA, ba), (idxB, bb))):
                s_d = gmap.get((t, h))
                if s_d is None:
                    continue
                s, d = s_d
                k = len(s)
                loc = s - (cfg.half if h else 0)
                idx[t, :k] = loc
                boff = 0 if h == 0 else ba
                for b in range(int(nv[t, h])):
                    sl = slice(b * TILE, min((b + 1) * TILE, k))
                    nvalid = sl.stop - sl.start
                    if nvalid > 0:
                        dloc[t, boff + b, :nvalid] = (d[sl] - t * TILE)
                    if nvalid < TILE:
                        dloc[t, boff + b, nvalid:] = 200.0
        # wrap indices: slot i -> [i % 16, i // 16], replicate x8 partitions
        def wrap(arr, cap):
            w = arr.reshape(nt, cap * 8, 16).transpose(0, 2, 1)  # [nt,16,cap*8]
            w = np.concatenate([w[t] for t in range(nt)], axis=1)  # [16, nt*cap*8]
            return np.tile(w, (8, 1)).astype(np.int16)
        idx_arrs.append((wrap(idxA, ba), wrap(idxB, bb)))
        # dstr: [128, nt*(ba+bb)]  value at [p, t*(ba+bb)+b] = dloc[t, b, p]
        dstr = dloc.reshape(nt * (ba + bb), TILE).T
        dstr_arrs.append(_f32(dstr))

    nv = nv.astype(int)
    return dict(nv=nv, ba=ba, bb=bb, nblk=nblk,
                idx=idx_arrs, dstr=dstr_arrs)


# ---------------------------------------------------------------------------
# program builder
# ---------------------------------------------------------------------------

def build_program(cfg, nv, ba, bb, debug=False, reps=1):
    nt = cfg.ntiles
    nc = bacc.Bacc("TRN2")

    # ---- external inputs --------------------------------------------------
    ein = {}

    def inp(name, shape, dt):
        ein[name] = nc.dram_tensor(name, list(shape), dt, kind="ExternalInput")
        return ein[name]

    x_own = inp("x_own", (cfg.nper, IN), F32)
    idxA = inp("idxA", (128, nt * ba * 8), I16)
    idxB = inp("idxB", (128, nt * bb * 8), I16)
    dstr = inp("dstr", (128, nt * (ba + bb)), F32)
    w_in, lw_in, atts_in, attd_in, cb_in, lb_in = [], [], [], [], [], []
    for L, (fin, fout) in enumerate(((IN, D1), (D1, D1), (D1, H))):
        kk = fin // 128
        w_in.append(inp(f"W{L}", (128, kk, fout), BF16))
        lout = fout if L < 2 else 1
        lw_in.append(inp(f"lW{L}", (128, kk, lout), BF16))
        aw = fout
        atts_in.append(inp(f"attS{L}", (128, aw), F32))
        attd_in.append(inp(f"attD{L}", (128, aw), F32))
        cw = D1 if L < 2 else 1
        cb_in.append(inp(f"cb{L}", (128, cw), F32))
        lb_in.append(inp(f"lb{L}", (128, cw), F32))
    iota_in = inp("iota", (128, 128), BF16)
    identb_in = inp("identb", (128, 128), BF16)
    identf_in = inp("identf", (128, 128), F32)

    out_ext = nc.dram_tensor("out", [cfg.nper, 1], F32, kind="ExternalOutput")

    # table widths per layer
    TW = [384, 384, 128]     # row stride (bf16 elems)
    RW = [260, 260, 8]       # scatter matmul width (msg + ones)
    XW = [256, 256, 4]       # xl width
    HI0 = [260, 260, 8]      # as_hi col
    LO0 = [264, 264, 12]
    USED = [268, 268, 16]

    cc_groups = [list(range(cfg.ncores))]

    with tile.TileContext(nc) as tc:
        import contextlib
        stack = contextlib.ExitStack()
        sb = stack.enter_context(tc.tile_pool(name="sb", bufs=3))
        sbc = stack.enter_context(tc.tile_pool(name="sbc", bufs=1))
        ps_big = stack.enter_context(tc.tile_pool(name="psbig", bufs=3, space="PSUM"))
        ps_sm = stack.enter_context(tc.tile_pool(name="pssm", bufs=2, space="PSUM"))
        ps_agg = stack.enter_context(tc.tile_pool(name="psagg", bufs=2, space="PSUM"))
        dram = stack.enter_context(tc.tile_pool(name="dram", bufs=1, space="DRAM"))

        nc.gpsimd.load_library(mlp_lib)

        # ---- persistent SBUF constants -----------------------------------
        def load_const(name, ap):
            t = sbc.tile(list(ap.shape), ap.dtype, name=name)
            nc.sync.dma_start(t[:], ap[:])
            return t

        W_sb = [load_const(f"W{L}sb", w_in[L]) for L in range(3)]
        lW_sb = [load_const(f"lW{L}sb", lw_in[L]) for L in range(3)]
        attS_sb = [load_const(f"attS{L}sb", atts_in[L]) for L in range(3)]
        attD_sb = [load_const(f"attD{L}sb", attd_in[L]) for L in range(3)]
        iota_sb = load_const("iotasb", iota_in)
        identb_sb = load_const("identbsb", identb_in)
        identf_sb = load_const("identfsb", identf_in)
        idxA_sb = load_const("idxAsb", idxA)
        idxB_sb = load_const("idxBsb", idxB)
        dstr_sb = load_const("dstrsb", dstr)
        cbl_sb = []
        for L in range(3):
            cw = D1 if L < 2 else 1
            cbt = sbc.tile([128, cw], F32, name=f"cbl{L}")
            c0 = load_const(f"cb{L}t", cb_in[L])
            l0 = load_const(f"lb{L}t", lb_in[L])
            nc.vector.tensor_add(cbt[:], c0[:], l0[:])
            cbl_sb.append(cbt)
        # per-tile ad storage (bf16) for all layers' current use
        ad_all = sbc.tile([128, nt * H], BF16, name="adall")

        # ---- DRAM scratch -------------------------------------------------
        t_own = [dram.tile([cfg.nper, TW[L]], BF16, name=f"town{L}") for L in range(3)]
        t_full = [dram.tile([cfg.npad, TW[L]], BF16, name=f"tfull{L}",
                            addr_space="Shared") for L in range(3)]
        if debug:
            h_own = [nc.dram_tensor(f"hdbg{L}", [cfg.nper, D1], BF16,
                                    kind="ExternalOutput") for L in range(2)]
        else:
            h_own = [dram.tile([cfg.nper, D1], BF16, name=f"hown{L}") for L in range(2)]
        lin_dram = dram.tile([cfg.nper, D1], F32, name="lindram")
        lin2_dram = dram.tile([cfg.nper, 1], F32, name="lin2dram")

        # ------------------------------------------------------------------
        def phase_a(L):
            fin = IN if L == 0 else D1
            kk = fin // 128
            fout = XW[L]
            lout = D1 if L < 2 else 1
            for t in range(nt):
                rows = slice(t * TILE, (t + 1) * TILE)
                if L == 0:
                    xin = sb.tile([128, IN], F32, tag="axin")
                    nc.sync.dma_start(xin[:], x_own[rows, :])
                    xt_ps = ps_big.tile([128, 128], F32, tag="bigps")
                    nc.tensor.transpose(xt_ps[:], xin[:], identf_sb[:])
                    actT = sb.tile([128, 1, 128], BF16, tag="actT")
                    nc.vector.tensor_copy(actT[:, 0, :], xt_ps[:])
                else:
                    hsrc = h_own[L - 1]
                    actT = sb.tile([128, 2, 128], BF16, tag="actT")
                    for k in range(2):
                        nc.sync.dma_start(
                            actT[:, k, :],
                            hsrc[rows, k * 128:(k + 1) * 128],
                            transpose=True,
                        )
                xl_ps = ps_big.tile([128, fout], F32, tag="bigps")
                lin_ps = ps_sm.tile([128, lout], F32, tag="smps")
                for k in range(kk):
                    nc.tensor.matmul(xl_ps[:], lhsT=actT[:, k, :], rhs=W_sb[L][:, k, :],
                                     start=(k == 0), stop=(k == kk - 1))
                for k in range(kk):
                    nc.tensor.matmul(lin_ps[:], lhsT=actT[:, k, :], rhs=lW_sb[L][:, k, :],
                                     start=(k == 0), stop=(k == kk - 1))
                # attention terms
                aa = sb.tile([128, 2 * H], F32, tag="aa")
                if L < 2:
                    for j, att in enumerate((attS_sb[L], attD_sb[L])):
                        prod = sb.tile([128, fout], F32, tag="prod")
                        nc.vector.tensor_mul(prod[:], xl_ps[:], att[:])
                        nc.vector.tensor_reduce(
                            aa[:, j * H:(j + 1) * H],
                            prod[:].rearrange("p (h j) -> p h j", h=H),
                            axis=mybir.AxisListType.X, op=mybir.AluOpType.add)
                else:
                    nc.vector.tensor_mul(aa[:, 0:H], xl_ps[:], attS_sb[L][:])
                    nc.vector.tensor_mul(aa[:, H:2 * H], xl_ps[:], attD_sb[L][:])
                nc.vector.tensor_copy(ad_all[:, t * H:(t + 1) * H], aa[:, H:2 * H])
                # table tile (full row width; pad cols zeroed for the
                # allgather + gather path)
                tbl = sb.tile([128, TW[L]], BF16, tag="tbl")
                nc.vector.memset(tbl[:, USED[L]:TW[L]], 0.0)
                if L < 2:
                    nc.vector.tensor_copy(
                        tbl[:, 0:XW[L]].rearrange("p (j h) -> p j h", h=H),
                        xl_ps[:].rearrange("p (h j) -> p j h", j=64),
                    )
                else:
                    nc.vector.tensor_copy(tbl[:, 0:H], xl_ps[:])
                nc.vector.memset(tbl[:, XW[L]:XW[L] + H], 1.0)
                # as hi/lo
                nc.scalar.activation(tbl[:, HI0[L]:HI0[L] + H], aa[:, 0:H],
                                     mybir.ActivationFunctionType.Copy)
                hi_f = sb.tile([128, H], F32, tag="hif")
                nc.scalar.activation(hi_f[:], tbl[:, HI0[L]:HI0[L] + H],
                                     mybir.ActivationFunctionType.Copy)
                nc.vector.tensor_sub(tbl[:, LO0[L]:LO0[L] + H], aa[:, 0:H], hi_f[:])
                nc.sync.dma_start(t_own[L][rows, :], tbl[:])
                # lin
                lin_sb = sb.tile([128, lout], F32, tag="linsb")
                nc.vector.tensor_copy(lin_sb[:], lin_ps[:])
                tgt = lin_dram if L < 2 else lin2_dram
                nc.sync.dma_start(tgt[rows, :], lin_sb[:])
            # publish table
            nc.gpsimd.collective_compute(
                "AllGather", mybir.AluOpType.bypass, replica_groups=cc_groups,
                ins=[t_own[L][:, :].opt()], outs=[t_full[L][:, :].opt()])

        # ------------------------------------------------------------------
        def edge_phase(L):
            tw, rw, used = TW[L], RW[L], USED[L]
            for t in range(nt):
                agg = ps_agg.tile([128, rw], F32, tag="agg")
                n_mm = int(nv[t, 0]) + int(nv[t, 1])
                mm_i = 0
                for hh, (idx_sb, cap, boff) in enumerate(
                        ((idxA_sb, ba, 0), (idxB_sb, bb, ba))):
                    v = int(nv[t, hh])
                    if v == 0:
                        continue
                    rlo = hh * cfg.half
                    gath = sb.tile([128, cap, tw], BF16, tag=f"gath{L % 2}")
                    nc.gpsimd.dma_gather(
                        gath[:, 0:v, :],
                        t_full[L][rlo:rlo + cfg.half, :],
                        idx_sb[:, t * cap * 8: t * cap * 8 + v * 8],
                        v * TILE, v * TILE, tw, single_packet=False)
                    # one-hot P and its transpose
                    p_all = sb.tile([128, cap, 128], BF16, tag="pall")
                    pt_sb = sb.tile([128, cap, 128], BF16, tag="ptsb")
                    blk0 = t * (ba + bb) + boff
                    for g in range(0, v, 4):
                        nb = min(4, v - g)
                        pt_ps = ps_big.tile([128, 512], BF16, tag="bigps")
                        for b in range(g, g + nb):
                            nc.vector.tensor_scalar(
                                out=p_all[:, b, :], in0=iota_sb[:],
                                scalar1=dstr_sb[:, blk0 + b: blk0 + b + 1],
                                scalar2=None, op0=mybir.AluOpType.is_equal)
                            nc.tensor.transpose(
                                pt_ps[:, (b - g) * 128:(b - g + 1) * 128],
                                p_all[:, b, :], identb_sb[:])
                        nc.scalar.activation(
                            pt_sb[:].rearrange("p c j -> p (c j)")[:, g * 128:(g + nb) * 128],
                            pt_ps[:, 0:nb * 128],
                            mybir.ActivationFunctionType.Copy)
                    # per-edge ad via PT @ ad_tile
                    ad_ps = ps_sm.tile([128, cap * H], F32, tag="smps")
                    for b in range(v):
                        nc.tensor.matmul(
                            ad_ps[:, b * H:(b + 1) * H], lhsT=pt_sb[:, b, :],
                            rhs=ad_all[:, t * H:(t + 1) * H],
                            start=True, stop=True)
                    # logits -> w (bf16)
                    e_sb = sb.tile([128, cap * H], F32, tag="esb")
                    hi_ap = gath[:, 0:v, HI0[L]:HI0[L] + H]
                    lo_ap = gath[:, 0:v, LO0[L]:LO0[L] + H]
                    nc.vector.tensor_add(
                        e_sb[:, 0:v * H].rearrange("p (c h) -> p c h", h=H),
                        hi_ap, lo_ap)
                    nc.vector.scalar_tensor_tensor(
                        out=e_sb[:, 0:v * H], in0=e_sb[:, 0:v * H], scalar=1.0,
                        in1=ad_ps[:, 0:v * H],
                        op0=mybir.AluOpType.mult, op1=mybir.AluOpType.add)
                    lr_sb = sb.tile([128, cap * H], F32, tag="lrsb")
                    nc.vector.tensor_scalar(
                        out=lr_sb[:, 0:v * H], in0=e_sb[:, 0:v * H],
                        scalar1=NEG, scalar2=None, op0=mybir.AluOpType.mult)
                    nc.vector.tensor_max(lr_sb[:, 0:v * H], lr_sb[:, 0:v * H],
                                         e_sb[:, 0:v * H])
                    w_bf = sb.tile([128, cap, H], BF16, tag="wbf")
                    nc.scalar.activation(
                        w_bf[:, 0:v, :].rearrange("p c h -> p (c h)"),
                        lr_sb[:, 0:v * H], mybir.ActivationFunctionType.Exp)
                    # msg + scatter
                    for g in range(0, v, 4):
                        nb = min(4, v - g)
                        msg = sb.tile([128, 4, rw], BF16, tag="msg")
                        w_ap = bass.AP(
                            w_bf.tensor, w_bf[:, g, :].offset,
                            [w_bf[:].ap[0], [H, nb], [0, rw // H], [1, H]],
                        )
                        nc.vector.tensor_tensor(
                            out=msg[:, 0:nb, :], in0=gath[:, g:g + nb, 0:rw],
                            in1=w_ap, op=mybir.AluOpType.mult)
                        for b in range(nb):
                            nc.tensor.matmul(
                                agg[:], lhsT=p_all[:, g + b, :], rhs=msg[:, b, :],
                                start=(mm_i == 0), stop=(mm_i == n_mm - 1),
                                skip_group_check=True)
                            mm_i += 1
                # ---- tile close ----
                rows = slice(t * TILE, (t + 1) * TILE)
                den_b = sb.tile([128, H], F32, tag="denb")
                nc.vector.tensor_scalar_add(den_b[:], agg[:, XW[L]:XW[L] + H], 1e-12)
                recip = sb.tile([128, H], F32, tag="recip")
                nc.vector.reciprocal(recip[:], den_b[:])
                if L < 2:
                    lin_sb = sb.tile([128, D1], F32, tag="linrd")
                    nc.sync.dma_start(lin_sb[:], lin_dram[rows, :])
                    gat = sb.tile([128, D1], F32, tag="gatsb")
                    # out head-major <- agg head-minor, scaled by recip
                    agg_hm = bass.AP(agg.tensor, agg[:].offset,
                                     [agg[:].ap[0], [1, H], [H, 64]])
                    rec_b = bass.AP(recip.tensor, recip[:].offset,
                                    [recip[:].ap[0], [1, H], [0, 64]])
                    nc.vector.tensor_tensor(out=gat[:].rearrange("p (h j) -> p h j", h=H),
                                            in0=agg_hm, in1=rec_b,
                                            op=mybir.AluOpType.mult)
                    gsum = sb.tile([128, D1], F32, tag="gsum")
                    nc.vector.scalar_tensor_tensor(
                        out=gsum[:], in0=gat[:], scalar=1.0, in1=lin_sb[:],
                        op0=mybir.AluOpType.mult, op1=mybir.AluOpType.add)
                    nc.vector.tensor_add(gsum[:], gsum[:], cbl_sb[L][:])
                    # ELU: relu(g) + exp(-relu(-g)) - 1
                    r1 = sb.tile([128, D1], F32, tag="r1")
                    nc.scalar.activation(r1[:], gsum[:],
                                         mybir.ActivationFunctionType.Relu)
                    r2 = sb.tile([128, D1], F32, tag="r2")
                    nc.scalar.activation(r2[:], gsum[:],
                                         mybir.ActivationFunctionType.Relu,
                                         scale=-1.0)
                    r3 = sb.tile([128, D1], F32, tag="r3")
                    nc.scalar.activation(r3[:], r2[:],
                                         mybir.ActivationFunctionType.Exp,
                                         scale=-1.0)
                    h_bf = sb.tile([128, D1], BF16, tag="hbf")
                    nc.vector.scalar_tensor_tensor(
                        out=h_bf[:], in0=r1[:], scalar=-1.0, in1=r3[:],
                        op0=mybir.AluOpType.add, op1=mybir.AluOpType.add)
                    nc.sync.dma_start(h_own[L][rows, :], h_bf[:])
                else:
                    lin_sb = sb.tile([128, 1], F32, tag="linrd2")
                    nc.sync.dma_start(lin_sb[:], lin2_dram[rows, :])
                    q = sb.tile([128, H], F32, tag="qsb")
                    nc.vector.tensor_mul(q[:], agg[:, 0:H], recip[:])
                    msum = sb.tile([128, 1], F32, tag="msum")
                    nc.vector.tensor_reduce(msum[:], q[:],
                                            axis=mybir.AxisListType.X,
                                            op=mybir.AluOpType.add)
                    o1 = sb.tile([128, 1], F32, tag="o1")
                    nc.vector.scalar_tensor_tensor(
                        out=o1[:], in0=msum[:], scalar=1.0 / H, in1=lin_sb[:],
                        op0=mybir.AluOpType.mult, op1=mybir.AluOpType.add)
                    nc.vector.tensor_add(o1[:], o1[:], cbl_sb[L][:])
                    nc.sync.dma_start(out_ext[rows, :], o1[:])

        for _rep in range(reps):
            for L in range(3):
                phase_a(L)
                edge_phase(L)

        stack.close()

    nc.compile()
    return nc, ein


# ---------------------------------------------------------------------------
# host wrapper
# ---------------------------------------------------------------------------

def _make_inputs(cfg, prep, core, x, weights):
    (cW0, cas0, cad0, cb0, lW0, lb0,
     cW1, cas1, cad1, cb1, lW1, lb1,
     cW2, cas2, cad2, cb2, lW2, lb2) = weights
    nper = cfg.nper
    xp = np.zeros((nper, IN), np.float32)
    lo = core * nper
    hi = min((core + 1) * nper, cfg.n_real)
    if hi > lo:
        xp[0:hi - lo] = x[lo:hi]
    idxA, idxB = prep["idx"][core]

    def stackk(w):
        w = _f32(w)
        kk = w.shape[0] // 128
        return _bf(np.stack([w[k * 128:(k + 1) * 128] for k in range(kk)], axis=1))

    rep = lambda v: np.broadcast_to(_f32(v).reshape(1, -1), (128, _f32(v).size)).copy()
    d = {
        "x_own": xp,
        "idxA": idxA, "idxB": idxB, "dstr": prep["dstr"][core],
        "W0": stackk(cW0), "W1": stackk(cW1), "W2": stackk(cW2),
        "lW0": stackk(lW0), "lW1": stackk(lW1), "lW2": stackk(lW2),
        "attS0": rep(cas0.reshape(-1)), "attD0": rep(cad0.reshape(-1)),
        "attS1": rep(cas1.reshape(-1)), "attD1": rep(cad1.reshape(-1)),
        "attS2": rep(cas2.reshape(-1)), "attD2": rep(cad2.reshape(-1)),
        "cb0": rep(cb0), "lb0": rep(lb0),
        "cb1": rep(cb1), "lb1": rep(lb1),
        "cb2": rep(cb2), "lb2": rep(lb2),
        "iota": _bf(np.broadcast_to(np.arange(128, dtype=np.float32), (128, 128))),
        "identb": _bf(np.eye(128, dtype=np.float32)),
        "identf": _f32(np.eye(128, dtype=np.float32)),
    }
    return d


_CACHE = {}
LAST_EXEC_NS = None
LAST_PROFILE = None


def kernel(x, edge_index, batch,
           cW0, cas0, cad0, cb0, lW0, lb0,
           cW1, cas1, cad1, cb1, lW1, lb1,
           cW2, cas2, cad2, cb2, lW2, lb2):
    cfg = Cfg(n_real=x.shape[0], ncores=8,
              nper=math.ceil(x.shape[0] / 8 / TILE) * TILE)
    x = _f32(x)
    edge_index = np.asarray(edge_index)
    prep = prep_graph(edge_index, cfg)
    key = ("prog", cfg.n_real, prep["ba"], prep["bb"], prep["nv"].tobytes())
    if key not in _CACHE:
        _CACHE[key] = build_program(cfg, prep["nv"], prep["ba"], prep["bb"])
    nc, _ = _CACHE[key]
    weights = (cW0, cas0, cad0, cb0, lW0, lb0,
               cW1, cas1, cad1, cb1, lW1, lb1,
               cW2, cas2, cad2, cb2, lW2, lb2)
    in_maps = [_make_inputs(cfg, prep, c, x, weights) for c in range(cfg.ncores)]
    import os
    trace = bool(os.environ.get("KERNEL_PROFILE"))
    res = run_bass_kernel_spmd(nc, in_maps, list(range(cfg.ncores)), trace=trace)
    global LAST_EXEC_NS, LAST_PROFILE
    LAST_EXEC_NS = getattr(res, "exec_time_ns", None)
    LAST_PROFILE = getattr(res, "profile_json", None)
    out = np.concatenate([res.results[c]["out"] for c in range(cfg.ncores)], axis=0)
    return out[:cfg.n_real].astype(np.float32)
